# revision 1
# baseline (speedup 1.0000x reference)
"""Self-contained Trainium2 Bass kernel for sparse attention.

Sharding: 8 cores = (image b, L-half). Each core receives its image's x0
ROTATED so its own 4096 rows come first (gather indices are remapped on
the host to match). The core computes LN+K/V for all 8192 rows, writes
packed bf16 [k|v] rows to DRAM scratch, then per 128-row tile gathers
2048 neighbor rows with dma_gather and runs attention + merge + MLP +
LN2 fully on-chip. No collectives.
"""
import numpy as np
import ml_dtypes

import concourse.bass as bass
import concourse.tile as tile
from concourse import bacc, mybir

F32 = mybir.dt.float32
BF16 = mybir.dt.bfloat16
I16 = mybir.dt.int16
I32 = mybir.dt.int32
AX = mybir.AxisListType
OP = mybir.AluOpType
AF = mybir.ActivationFunctionType
ts = bass.ts

L, C, NJ, NH, HD = 8192, 128, 16, 8, 16
LH = L // 2            # rows computed per core
NT_FULL = L // 128     # 64 k/v tiles
NT_HALF = LH // 128    # 32 attention tiles
EPS = 1e-5


def build_nc(nontrivial_ln1: bool, nontrivial_ln2: bool):
    nc = bacc.Bacc(None, target_bir_lowering=False, debug=False)

    x0f = nc.declare_dram_parameter("x0f", [L, C], F32, isOutput=False)
    gidx = nc.declare_dram_parameter("gidx", [128, NT_HALF * NJ], I32, isOutput=False)
    wnames = ["wq", "wk", "wv", "wm", "w1", "w2"]
    wparams = {n: nc.declare_dram_parameter(n, [C, C], BF16, isOutput=False) for n in wnames}
    identp = nc.declare_dram_parameter("ident", [C, C], BF16, isOutput=False)
    if nontrivial_ln1:
        bqkv = nc.declare_dram_parameter("bqkv", [1, 3 * C], F32, isOutput=False)
    if nontrivial_ln2:
        g2b2 = nc.declare_dram_parameter("g2b2", [1, 2 * C], F32, isOutput=False)
    out = nc.declare_dram_parameter("out", [LH, C], F32, isOutput=True)

    with tile.TileContext(nc) as tc:
        with (
            tc.tile_pool(name="res", bufs=1) as res,
            tc.tile_pool(name="dram", bufs=1, space="DRAM") as dram,
        ):
            kv_dram = dram.tile([L, 2 * C], BF16)
            x0_res = res.tile([128, NT_HALF * 128], F32)   # our-half x0 tiles
            q_res = res.tile([128, NT_HALF * 128], BF16)   # our-half q tiles
            idx_res = res.tile([128, NT_HALF * NJ], I32)
            ident = res.tile([128, 128], BF16)
            wsb = {n: res.tile([C, C], BF16, name=f"w_{n}", tag=f"w_{n}") for n in wnames}

            nc.sync.dma_start(idx_res[:], gidx[:])
            for n in wnames:
                nc.sync.dma_start(wsb[n][:], wparams[n][:])
            nc.sync.dma_start(ident[:], identp[:])
            if nontrivial_ln1:
                bqkv_sb = res.tile([1, 3 * C], F32)
                nc.sync.dma_start(bqkv_sb[:], bqkv[:])
            if nontrivial_ln2:
                g2b2_sb = res.tile([1, 2 * C], F32)
                nc.sync.dma_start(g2b2_sb[:], g2b2[:])

            # ---------------- Phase 1: LN1 + K/V (+Q) projections ----------------
            with (
                tc.tile_pool(name="p1", bufs=3) as p1,
                tc.tile_pool(name="p1s", bufs=2) as p1s,
                tc.tile_pool(name="ps1", bufs=2, space="PSUM") as ps1,
            ):
                for t in range(NT_FULL):
                    ours = t < NT_HALF
                    if ours:
                        x0t = x0_res[:, ts(t, 128)]
                    else:
                        x0t_tile = p1.tile([128, 128], F32, tag="x0t")
                        x0t = x0t_tile[:]
                    nc.sync.dma_start(x0t, x0f[ts(t, 128), :])

                    # LN1 stats
                    ssum = p1s.tile([128, 1], F32, tag="ssum")
                    mu = p1s.tile([128, 1], F32, tag="mu")
                    sq = p1s.tile([128, 128], F32, tag="sq")
                    m2 = p1s.tile([128, 1], F32, tag="m2")
                    var = p1s.tile([128, 1], F32, tag="var")
                    std = p1s.tile([128, 1], F32, tag="std")
                    rstd = p1s.tile([128, 1], F32, tag="rstd")
                    xn = p1.tile([128, 128], BF16, tag="xn")
                    nc.scalar.activation(sq[:], x0t, AF.Square, accum_out=m2[:])
                    nc.scalar.activation(sqc := p1s.tile([128, 128], BF16, tag="sqc", name="sqc"),
                                         x0t, AF.Copy, accum_out=ssum[:])
                    nc.vector.tensor_scalar_mul(mu[:], ssum[:], 1.0 / C)
                    nc.vector.tensor_scalar(
                        var[:], mu[:], scalar1=mu[:], scalar2=None, op0=OP.mult
                    )
                    nc.vector.tensor_scalar(
                        m2s2 := p1s.tile([128, 1], F32, tag="m2s2", name="m2s2"),
                        m2[:], scalar1=1.0 / C, scalar2=EPS, op0=OP.mult, op1=OP.add,
                    )
                    nc.vector.tensor_tensor(var[:], m2s2, var[:], op=OP.subtract)
                    nc.scalar.activation(std[:], var[:], AF.Sqrt)
                    nc.vector.reciprocal(rstd[:], std[:])
                    nc.vector.tensor_scalar(
                        xn[:], x0t, scalar1=mu[:], scalar2=rstd[:],
                        op0=OP.subtract, op1=OP.mult,
                    )

                    # transpose xn -> xnT (bf16)
                    xnT_ps = ps1.tile([128, 128], BF16, tag="xnT_ps")
                    xnT = p1.tile([128, 128], BF16, tag="xnT")
                    nc.tensor.transpose(xnT_ps[:], xn[:], ident[:])
                    nc.scalar.copy(xnT[:], xnT_ps[:])

                    # k/v projections -> packed [k|v] bf16 rows
                    kvt = p1.tile([128, 2 * C], BF16, tag="kvt")
                    k_ps = ps1.tile([128, 128], F32, tag="k_ps")
                    v_ps = ps1.tile([128, 128], F32, tag="v_ps")
                    nc.tensor.matmul(k_ps[:], lhsT=xnT[:], rhs=wsb["wk"][:], start=True, stop=True)
                    nc.tensor.matmul(v_ps[:], lhsT=xnT[:], rhs=wsb["wv"][:], start=True, stop=True)
                    if nontrivial_ln1:
                        nc.vector.tensor_tensor(
                            kvt[:, 0:C], k_ps[:],
                            bqkv_sb[:, C:2 * C].to_broadcast([128, C]), op=OP.add)
                        nc.vector.tensor_tensor(
                            kvt[:, C:2 * C], v_ps[:],
                            bqkv_sb[:, 2 * C:3 * C].to_broadcast([128, C]), op=OP.add)
                    else:
                        nc.scalar.copy(kvt[:, 0:C], k_ps[:])
                        nc.scalar.copy(kvt[:, C:2 * C], v_ps[:])
                    if ours:
                        q_ps = ps1.tile([128, 128], F32, tag="q_ps")
                        nc.tensor.matmul(q_ps[:], lhsT=xnT[:], rhs=wsb["wq"][:], start=True, stop=True)
                        if nontrivial_ln1:
                            nc.vector.tensor_tensor(
                                q_res[:, ts(t, 128)], q_ps[:],
                                bqkv_sb[:, 0:C].to_broadcast([128, C]), op=OP.add)
                        else:
                            nc.scalar.copy(q_res[:, ts(t, 128)], q_ps[:])
                    nc.sync.dma_start(kv_dram[ts(t, 128), :], kvt[:])

            # ---------------- Phase 2: gather + attention + MLP ----------------
            with (
                tc.tile_pool(name="p2", bufs=3) as p2,
                tc.tile_pool(name="p2s", bufs=2) as p2s,
                tc.tile_pool(name="ps2", bufs=1, space="PSUM") as ps2,
            ):
                for t in range(NT_HALF):
                    kvg = p2.tile([128, NJ * 2 * C], BF16, tag="kvg")
                    for j in range(NJ):
                        nc.gpsimd.indirect_dma_start(
                            out=kvg[:, ts(j, 2 * C)],
                            out_offset=None,
                            in_=kv_dram[:],
                            in_offset=bass.IndirectOffsetOnAxis(
                                ap=idx_res[:, t * NJ + j:t * NJ + j + 1], axis=0),
                        )
                    kvg_j = kvg[:].rearrange("p (j x) -> p j x", j=NJ)

                    # qk = sum_d q*kg per (j, head)
                    prod = p2s.tile([128, NJ * C], BF16, tag="prod")
                    qk = p2s.tile([128, NJ * NH], F32, tag="qk")
                    nc.vector.tensor_tensor(
                        prod[:].rearrange("p (j c) -> p j c", j=NJ),
                        q_res[:, ts(t, 128)].unsqueeze(1).to_broadcast([128, NJ, C]),
                        kvg_j[:, :, 0:C],
                        op=OP.mult,
                    )
                    tr8 = p2s.tile([128, NJ * NH * 8], BF16, tag="tr8")
                    tr4 = p2s.tile([128, NJ * NH * 4], BF16, tag="tr4")
                    tr2 = p2s.tile([128, NJ * NH * 2], BF16, tag="tr2")
                    p4d = prod[:].rearrange("p (j h d) -> p j h d", j=NJ, h=NH)
                    t8 = tr8[:].rearrange("p (j h d) -> p j h d", j=NJ, h=NH)
                    t4 = tr4[:].rearrange("p (j h d) -> p j h d", j=NJ, h=NH)
                    t2 = tr2[:].rearrange("p (j h d) -> p j h d", j=NJ, h=NH)
                    nc.vector.tensor_tensor(t8, p4d[:, :, :, 0:8], p4d[:, :, :, 8:16], op=OP.add)
                    nc.vector.tensor_tensor(t4, t8[:, :, :, 0:4], t8[:, :, :, 4:8], op=OP.add)
                    nc.vector.tensor_tensor(t2, t4[:, :, :, 0:2], t4[:, :, :, 2:4], op=OP.add)
                    nc.vector.tensor_tensor(
                        qk[:].rearrange("p (j h) -> p j h", j=NJ, h=NH).unsqueeze(3),
                        t2[:, :, :, 0:1], t2[:, :, :, 1:2], op=OP.add)
                    # softmax over j (no max subtraction; |qk| <~ 6)
                    E = p2s.tile([128, NJ * NH], BF16, tag="E")
                    sE = p2s.tile([128, NH], F32, tag="sE")
                    rec = p2s.tile([128, NH], F32, tag="rec")
                    A = p2s.tile([128, NJ * NH], BF16, tag="A")
                    nc.scalar.activation(E[:], qk[:], AF.Exp)
                    nc.vector.tensor_reduce(
                        sE[:], E[:].rearrange("p (j h) -> p h j", j=NJ), axis=AX.X, op=OP.add
                    )
                    nc.vector.reciprocal(rec[:], sE[:])
                    nc.vector.tensor_tensor(
                        A[:].rearrange("p (j h) -> p j h", j=NJ),
                        E[:].rearrange("p (j h) -> p j h", j=NJ),
                        rec[:].unsqueeze(1).to_broadcast([128, NJ, NH]),
                        op=OP.mult,
                    )
                    # att = sum_j A * vg
                    prod2 = p2s.tile([128, NJ * C], BF16, tag="prod2")
                    att = p2s.tile([128, C], BF16, tag="att")
                    Aexp = p2s.tile([128, NJ * C], BF16, tag="Aexp")
                    nc.scalar.activation(
                        Aexp[:].rearrange("p (j h d) -> p j h d", j=NJ, h=NH),
                        A[:].rearrange("p (j h) -> p j h", j=NJ).unsqueeze(3).to_broadcast([128, NJ, NH, HD]),
                        AF.Copy)
                    nc.vector.tensor_tensor(
                        prod2[:].rearrange("p (j c) -> p j c", j=NJ),
                        kvg_j[:, :, C:2 * C],
                        Aexp[:].rearrange("p (j c) -> p j c", j=NJ),
                        op=OP.mult,
                    )
                    av8 = p2s.tile([128, 8 * C], BF16, tag="av8")
                    av4 = p2s.tile([128, 4 * C], BF16, tag="av4")
                    av2 = p2s.tile([128, 2 * C], BF16, tag="av2")
                    nc.vector.tensor_tensor(av8[:], prod2[:, 0:8 * C], prod2[:, 8 * C:16 * C], op=OP.add)
                    nc.vector.tensor_tensor(av4[:], av8[:, 0:4 * C], av8[:, 4 * C:8 * C], op=OP.add)
                    nc.vector.tensor_tensor(av2[:], av4[:, 0:2 * C], av4[:, 2 * C:4 * C], op=OP.add)
                    nc.vector.tensor_tensor(att[:], av2[:, 0:C], av2[:, C:2 * C], op=OP.add)

                    # merge: qv = att @ Wm.T ; message = x0 + qv
                    attT_ps = ps2.tile([128, 128], BF16, tag="attT_ps")
                    attT = p2s.tile([128, 128], BF16, tag="attT")
                    nc.tensor.transpose(attT_ps[:], att[:], ident[:])
                    nc.scalar.copy(attT[:], attT_ps[:])
                    qv_ps = ps2.tile([128, 128], F32, tag="qv_ps")
                    nc.tensor.matmul(qv_ps[:], lhsT=attT[:], rhs=wsb["wm"][:], start=True, stop=True)
                    msg = p2s.tile([128, 128], BF16, tag="msg")
                    nc.vector.tensor_tensor(msg[:], x0_res[:, ts(t, 128)], qv_ps[:], op=OP.add)

                    # mlp
                    msgT_ps = ps2.tile([128, 128], BF16, tag="msgT_ps")
                    msgT = p2s.tile([128, 128], BF16, tag="msgT")
                    nc.tensor.transpose(msgT_ps[:], msg[:], ident[:])
                    nc.scalar.copy(msgT[:], msgT_ps[:])
                    m1_ps = ps2.tile([128, 128], F32, tag="m1_ps")
                    nc.tensor.matmul(m1_ps[:], lhsT=msgT[:], rhs=wsb["w1"][:], start=True, stop=True)
                    m1 = p2s.tile([128, 128], BF16, tag="m1")
                    nc.scalar.activation(m1[:], m1_ps[:], AF.Relu)
                    m1T_ps = ps2.tile([128, 128], BF16, tag="m1T_ps")
                    m1T = p2s.tile([128, 128], BF16, tag="m1T")
                    nc.tensor.transpose(m1T_ps[:], m1[:], ident[:])
                    nc.scalar.copy(m1T[:], m1T_ps[:])
                    m2_ps = ps2.tile([128, 128], F32, tag="m2_ps")
                    nc.tensor.matmul(m2_ps[:], lhsT=m1T[:], rhs=wsb["w2"][:], start=True, stop=True)

                    # LN2 + residual
                    m2sb = p2s.tile([128, 128], F32, tag="m2sb")
                    nc.scalar.activation(m2sb[:], m2_ps[:], AF.Copy)
                    ssum = p2s.tile([128, 1], F32, tag="ssum2")
                    mu = p2s.tile([128, 1], F32, tag="mu2")
                    sq = p2s.tile([128, 128], F32, tag="sq2")
                    m2st = p2s.tile([128, 1], F32, tag="m2st")
                    var = p2s.tile([128, 1], F32, tag="var2")
                    std = p2s.tile([128, 1], F32, tag="std2")
                    rstd = p2s.tile([128, 1], F32, tag="rstd2")
                    nc.scalar.activation(sq[:], m2sb[:], AF.Square, accum_out=m2st[:])
                    nc.scalar.activation(sqc2 := p2s.tile([128, 128], BF16, tag="sqc2", name="sqc2"),
                                         m2sb[:], AF.Copy, accum_out=ssum[:])
                    nc.vector.tensor_scalar_mul(mu[:], ssum[:], 1.0 / C)
                    nc.vector.tensor_scalar(
                        var[:], mu[:], scalar1=mu[:], scalar2=None, op0=OP.mult
                    )
                    nc.vector.tensor_scalar(
                        m2s2b := p2s.tile([128, 1], F32, tag="m2s2b", name="m2s2b"),
                        m2st[:], scalar1=1.0 / C, scalar2=EPS, op0=OP.mult, op1=OP.add,
                    )
                    nc.vector.tensor_tensor(var[:], m2s2b, var[:], op=OP.subtract)
                    nc.scalar.activation(std[:], var[:], AF.Sqrt)
                    nc.vector.reciprocal(rstd[:], std[:])
                    zn = p2s.tile([128, 128], F32, tag="zn")
                    outt = p2s.tile([128, 128], F32, tag="outt")
                    nc.vector.tensor_scalar(
                        zn[:], m2sb[:], scalar1=mu[:], scalar2=rstd[:],
                        op0=OP.subtract, op1=OP.mult,
                    )
                    if nontrivial_ln2:
                        nc.vector.tensor_tensor(
                            zn[:], zn[:], g2b2_sb[:, 0:C].to_broadcast([128, C]), op=OP.mult)
                        nc.vector.tensor_tensor(
                            zn[:], zn[:], g2b2_sb[:, C:2 * C].to_broadcast([128, C]), op=OP.add)
                    nc.vector.tensor_tensor(outt[:], zn[:], x0_res[:, ts(t, 128)], op=OP.add)
                    nc.sync.dma_start(out[ts(t, 128), :], outt[:])

    nc.finalize()
    return nc


def prep_core_inputs(x0_img: np.ndarray, query_img: np.ndarray, half: int, w):
    """Host-side prep for one core. w: dict of raw f32 weights g1,b1,g2,b2,Wq..W2."""
    ofs = half * LH
    x0r = np.ascontiguousarray(np.roll(x0_img, -ofs, axis=0))
    lq = query_img[ofs:ofs + LH, :].astype(np.int64)
    lq = (lq - ofs) % L  # remap into rotated coordinates

    gidx = np.zeros((128, NT_HALF * NJ), np.int32)
    for t in range(NT_HALF):
        gidx[:, t * NJ:(t + 1) * NJ] = lq[t * 128:(t + 1) * 128, :]
    bf = ml_dtypes.bfloat16
    g1 = w["g1"]
    m = {
        "x0f": x0r,
        "gidx": gidx,
        "ident": np.eye(C, dtype=np.float32).astype(bf),
        "wq": np.ascontiguousarray((0.25 * w["Wq"] * g1[None, :]).T).astype(bf),
        "wk": np.ascontiguousarray((w["Wk"] * g1[None, :]).T).astype(bf),
        "wv": np.ascontiguousarray((w["Wv"] * g1[None, :]).T).astype(bf),
        "wm": np.ascontiguousarray(w["Wm"].T).astype(bf),
        "w1": np.ascontiguousarray(w["W1"].T).astype(bf),
        "w2": np.ascontiguousarray(w["W2"].T).astype(bf),
    }
    b1 = w["b1"]
    nontrivial_ln1 = bool(np.any(b1 != 0.0))
    if nontrivial_ln1:
        m["bqkv"] = np.concatenate(
            [0.25 * (w["Wq"] @ b1), w["Wk"] @ b1, w["Wv"] @ b1]
        ).reshape(1, 3 * C).astype(np.float32)
    nontrivial_ln2 = bool(np.any(w["g2"] != 1.0) or np.any(w["b2"] != 0.0))
    if nontrivial_ln2:
        m["g2b2"] = np.concatenate([w["g2"], w["b2"]]).reshape(1, 2 * C).astype(np.float32)
    return m, nontrivial_ln1, nontrivial_ln2


def kernel(**inputs):
    from concourse.bass_utils import run_bass_kernel_spmd

    x0 = np.asarray(inputs["x0"], np.float32)
    query = np.asarray(inputs["query"])
    w = {k: np.asarray(inputs[k], np.float32)
         for k in ["Wq", "Wk", "Wv", "Wm", "W1", "W2", "g1", "b1", "g2", "b2"]}
    B = x0.shape[0]

    in_maps = []
    nt1 = nt2 = False
    for c in range(8):
        b, half = c // 2, c % 2
        m, nt1, nt2 = prep_core_inputs(x0[b], np.asarray(query[b]), half, w)
        in_maps.append(m)

    nc = build_nc(nt1, nt2)
    res = run_bass_kernel_spmd(nc, in_maps, core_ids=list(range(8)))

    outp = np.empty((B, L, C), np.float32)
    for c in range(8):
        b, half = c // 2, c % 2
        outp[b, half * LH:(half + 1) * LH, :] = res.results[c]["out"]
    return outp



# revision 21
# speedup vs baseline: 1.0178x; 1.0178x over previous
"""Self-contained Trainium2 Bass kernel for sparse attention.

Sharding: 8 cores = (image b, L-half). Each core receives its image's x0
ROTATED so its own 4096 rows come first (gather indices are remapped on
the host to match). The core computes LN+K/V for all 8192 rows, writes
packed bf16 [k|v] rows to DRAM scratch, then per 128-row tile gathers
2048 neighbor rows with dma_gather and runs attention + merge + MLP +
LN2 fully on-chip. No collectives.
"""
import numpy as np
import ml_dtypes

import concourse.bass as bass
import concourse.tile as tile
from concourse import bacc, library_config, mybir

F32 = mybir.dt.float32
BF16 = mybir.dt.bfloat16
I16 = mybir.dt.int16
I32 = mybir.dt.int32
AX = mybir.AxisListType
OP = mybir.AluOpType
AF = mybir.ActivationFunctionType
ts = bass.ts

L, C, NJ, NH, HD = 8192, 128, 16, 8, 16
LH = L // 2            # rows computed per core
NT_FULL = L // 128     # 64 k/v tiles
NT_HALF = LH // 128    # 32 attention tiles
EPS = 1e-5


NSWQ = int(__import__("os").environ.get("NSWQ", "4"))


def build_nc(nontrivial_ln1: bool, nontrivial_ln2: bool):
    nc = bacc.Bacc(None, target_bir_lowering=False, debug=False,
                   num_swdge_queues=NSWQ)

    x0f = nc.declare_dram_parameter("x0f", [L, C], F32, isOutput=False)
    gidx32 = nc.declare_dram_parameter("gidx32", [128, NT_HALF * NJ], I32, isOutput=False)
    wnames = ["wq", "wk", "wv", "wm", "w1", "w2"]
    wparams = {n: nc.declare_dram_parameter(n, [C, C], BF16, isOutput=False) for n in wnames}
    identp = nc.declare_dram_parameter("ident", [C, C], BF16, isOutput=False)
    if nontrivial_ln1:
        bqkv = nc.declare_dram_parameter("bqkv", [1, 3 * C], F32, isOutput=False)
    if nontrivial_ln2:
        g2b2 = nc.declare_dram_parameter("g2b2", [1, 2 * C], F32, isOutput=False)
    out = nc.declare_dram_parameter("out", [LH, C], F32, isOutput=True)

    with tile.TileContext(nc) as tc:
        with (
            tc.tile_pool(name="res", bufs=1) as res,
            tc.tile_pool(name="dram", bufs=1, space="DRAM") as dram,
        ):
            kv_dram = dram.tile([L, 2 * C], BF16)
            x0_res = res.tile([128, NT_HALF * 128], F32)   # our-half x0 tiles
            q_res = res.tile([128, NT_HALF * 128], BF16)   # our-half q tiles
            idx32_res = res.tile([128, NT_HALF * NJ], I32)
            nc.sync.dma_start(idx32_res[:], gidx32[:])
            ident = res.tile([128, 128], BF16)
            wsb = {n: res.tile([C, C], BF16, name=f"w_{n}", tag=f"w_{n}") for n in wnames}

            for n in wnames:
                nc.sync.dma_start(wsb[n][:], wparams[n][:])
            nc.sync.dma_start(ident[:], identp[:])
            if nontrivial_ln1:
                bqkv_sb = res.tile([1, 3 * C], F32)
                nc.sync.dma_start(bqkv_sb[:], bqkv[:])
            if nontrivial_ln2:
                g2b2_sb = res.tile([1, 2 * C], F32)
                nc.sync.dma_start(g2b2_sb[:], g2b2[:])

            # ---------------- Phase 1: LN1 + K/V (+Q) projections ----------------
            with (
                tc.tile_pool(name="p1", bufs=3) as p1,
                tc.tile_pool(name="p1s", bufs=2) as p1s,
                tc.tile_pool(name="ps1", bufs=2, space="PSUM") as ps1,
            ):
                for t in range(NT_FULL):
                    ours = t < NT_HALF
                    if ours:
                        x0t = x0_res[:, ts(t, 128)]
                    else:
                        x0t_tile = p1.tile([128, 128], F32, tag="x0t")
                        x0t = x0t_tile[:]
                    nc.sync.dma_start(x0t, x0f[ts(t, 128), :])

                    # LN1 stats
                    ssum = p1s.tile([128, 1], F32, tag="ssum")
                    mu = p1s.tile([128, 1], F32, tag="mu")
                    sq = p1s.tile([128, 128], F32, tag="sq")
                    m2 = p1s.tile([128, 1], F32, tag="m2")
                    var = p1s.tile([128, 1], F32, tag="var")
                    std = p1s.tile([128, 1], F32, tag="std")
                    rstd = p1s.tile([128, 1], F32, tag="rstd")
                    xn = p1.tile([128, 128], BF16, tag="xn")
                    nc.scalar.activation(sq[:], x0t, AF.Square, accum_out=m2[:])
                    nc.scalar.activation(sqc := p1s.tile([128, 128], BF16, tag="sqc", name="sqc"),
                                         x0t, AF.Copy, accum_out=ssum[:])
                    nc.vector.tensor_scalar_mul(mu[:], ssum[:], 1.0 / C)
                    nc.vector.tensor_scalar(
                        var[:], mu[:], scalar1=mu[:], scalar2=None, op0=OP.mult
                    )
                    nc.vector.tensor_scalar(
                        m2s2 := p1s.tile([128, 1], F32, tag="m2s2", name="m2s2"),
                        m2[:], scalar1=1.0 / C, scalar2=EPS, op0=OP.mult, op1=OP.add,
                    )
                    nc.vector.tensor_tensor(var[:], m2s2, var[:], op=OP.subtract)
                    nc.scalar.activation(std[:], var[:], AF.Sqrt)
                    nc.vector.reciprocal(rstd[:], std[:])
                    nc.vector.tensor_scalar(
                        xn[:], x0t, scalar1=mu[:], scalar2=rstd[:],
                        op0=OP.subtract, op1=OP.mult,
                    )

                    # transpose xn -> xnT (bf16)
                    xnT_ps = ps1.tile([128, 128], BF16, tag="xnT_ps")
                    xnT = p1.tile([128, 128], BF16, tag="xnT")
                    nc.tensor.transpose(xnT_ps[:], xn[:], ident[:])
                    nc.scalar.copy(xnT[:], xnT_ps[:])

                    # k/v projections -> packed [k|v] bf16 rows
                    kvt = p1.tile([128, 2 * C], BF16, tag="kvt")
                    k_ps = ps1.tile([128, 128], F32, tag="k_ps")
                    v_ps = ps1.tile([128, 128], F32, tag="v_ps")
                    nc.tensor.matmul(k_ps[:], lhsT=xnT[:], rhs=wsb["wk"][:], start=True, stop=True)
                    nc.tensor.matmul(v_ps[:], lhsT=xnT[:], rhs=wsb["wv"][:], start=True, stop=True)
                    if nontrivial_ln1:
                        nc.vector.tensor_tensor(
                            kvt[:, 0:C], k_ps[:],
                            bqkv_sb[:, C:2 * C].to_broadcast([128, C]), op=OP.add)
                        nc.vector.tensor_tensor(
                            kvt[:, C:2 * C], v_ps[:],
                            bqkv_sb[:, 2 * C:3 * C].to_broadcast([128, C]), op=OP.add)
                    else:
                        nc.scalar.copy(kvt[:, 0:C], k_ps[:])
                        nc.scalar.copy(kvt[:, C:2 * C], v_ps[:])
                    if ours:
                        q_ps = ps1.tile([128, 128], F32, tag="q_ps")
                        nc.tensor.matmul(q_ps[:], lhsT=xnT[:], rhs=wsb["wq"][:], start=True, stop=True)
                        if nontrivial_ln1:
                            nc.vector.tensor_tensor(
                                q_res[:, ts(t, 128)], q_ps[:],
                                bqkv_sb[:, 0:C].to_broadcast([128, C]), op=OP.add)
                        else:
                            nc.scalar.copy(q_res[:, ts(t, 128)], q_ps[:])
                    nc.sync.dma_start(kv_dram[ts(t, 128), :], kvt[:])

            # ---------------- Phase 2: gather + attention + MLP ----------------
            with (
                tc.tile_pool(name="p2", bufs=3) as p2,
                tc.tile_pool(name="p2s", bufs=2) as p2s,
                tc.tile_pool(name="ps2", bufs=1, space="PSUM") as ps2,
            ):
                for t in range(NT_HALF):
                    kvg = p2.tile([128, NJ * 2 * C], BF16, tag="kvg")
                    for j in range(NJ):
                        gi = nc.gpsimd.indirect_dma_start(
                            out=kvg[:, ts(j, 2 * C)],
                            out_offset=None,
                            in_=kv_dram[:],
                            in_offset=bass.IndirectOffsetOnAxis(
                                ap=idx32_res[:, t * NJ + j:t * NJ + j + 1], axis=0),
                        )
                        q = (t * NJ + j) % NSWQ
                        if q:
                            gi.ins.queue = f"qPoolDynamic{q}"
                    kvg_j = kvg[:].rearrange("p (j x) -> p j x", j=NJ)

                    # qk = sum_d q*kg per (j, head)
                    prod = p2s.tile([128, NJ * C], BF16, tag="prod")
                    qk = p2s.tile([128, NJ * NH], F32, tag="qk")
                    nc.vector.tensor_tensor(
                        prod[:].rearrange("p (j c) -> p j c", j=NJ),
                        q_res[:, ts(t, 128)].unsqueeze(1).to_broadcast([128, NJ, C]),
                        kvg_j[:, :, 0:C],
                        op=OP.mult,
                    )
                    tr8 = p2s.tile([128, NJ * NH * 8], BF16, tag="tr8")
                    tr4 = p2s.tile([128, NJ * NH * 4], BF16, tag="tr4")
                    tr2 = p2s.tile([128, NJ * NH * 2], BF16, tag="tr2")
                    p4d = prod[:].rearrange("p (j h d) -> p j h d", j=NJ, h=NH)
                    t8 = tr8[:].rearrange("p (j h d) -> p j h d", j=NJ, h=NH)
                    t4 = tr4[:].rearrange("p (j h d) -> p j h d", j=NJ, h=NH)
                    t2 = tr2[:].rearrange("p (j h d) -> p j h d", j=NJ, h=NH)
                    nc.vector.tensor_tensor(t8, p4d[:, :, :, 0:8], p4d[:, :, :, 8:16], op=OP.add)
                    nc.vector.tensor_tensor(t4, t8[:, :, :, 0:4], t8[:, :, :, 4:8], op=OP.add)
                    nc.vector.tensor_tensor(t2, t4[:, :, :, 0:2], t4[:, :, :, 2:4], op=OP.add)
                    nc.vector.tensor_tensor(
                        qk[:].rearrange("p (j h) -> p j h", j=NJ, h=NH).unsqueeze(3),
                        t2[:, :, :, 0:1], t2[:, :, :, 1:2], op=OP.add)
                    # softmax over j (no max subtraction; |qk| <~ 6)
                    E = p2s.tile([128, NJ * NH], BF16, tag="E")
                    sE = p2s.tile([128, NH], F32, tag="sE")
                    rec = p2s.tile([128, NH], F32, tag="rec")
                    A = p2s.tile([128, NJ * NH], BF16, tag="A")
                    nc.scalar.activation(E[:], qk[:], AF.Exp)
                    nc.vector.tensor_reduce(
                        sE[:], E[:].rearrange("p (j h) -> p h j", j=NJ), axis=AX.X, op=OP.add
                    )
                    nc.vector.reciprocal(rec[:], sE[:])
                    nc.vector.tensor_tensor(
                        A[:].rearrange("p (j h) -> p j h", j=NJ),
                        E[:].rearrange("p (j h) -> p j h", j=NJ),
                        rec[:].unsqueeze(1).to_broadcast([128, NJ, NH]),
                        op=OP.mult,
                    )
                    # att = sum_j A * vg
                    prod2 = p2s.tile([128, NJ * C], BF16, tag="prod2")
                    att = p2s.tile([128, C], BF16, tag="att")
                    Aexp = p2s.tile([128, NJ * C], BF16, tag="Aexp")
                    nc.scalar.activation(
                        Aexp[:].rearrange("p (j h d) -> p j h d", j=NJ, h=NH),
                        A[:].rearrange("p (j h) -> p j h", j=NJ).unsqueeze(3).to_broadcast([128, NJ, NH, HD]),
                        AF.Copy)
                    nc.vector.tensor_tensor(
                        prod2[:].rearrange("p (j c) -> p j c", j=NJ),
                        kvg_j[:, :, C:2 * C],
                        Aexp[:].rearrange("p (j c) -> p j c", j=NJ),
                        op=OP.mult,
                    )
                    av8 = p2s.tile([128, 8 * C], BF16, tag="av8")
                    av4 = p2s.tile([128, 4 * C], BF16, tag="av4")
                    av2 = p2s.tile([128, 2 * C], BF16, tag="av2")
                    nc.vector.tensor_tensor(av8[:], prod2[:, 0:8 * C], prod2[:, 8 * C:16 * C], op=OP.add)
                    nc.vector.tensor_tensor(av4[:], av8[:, 0:4 * C], av8[:, 4 * C:8 * C], op=OP.add)
                    nc.vector.tensor_tensor(av2[:], av4[:, 0:2 * C], av4[:, 2 * C:4 * C], op=OP.add)
                    nc.vector.tensor_tensor(att[:], av2[:, 0:C], av2[:, C:2 * C], op=OP.add)

                    # merge: qv = att @ Wm.T ; message = x0 + qv
                    attT_ps = ps2.tile([128, 128], BF16, tag="attT_ps")
                    attT = p2s.tile([128, 128], BF16, tag="attT")
                    nc.tensor.transpose(attT_ps[:], att[:], ident[:])
                    nc.scalar.copy(attT[:], attT_ps[:])
                    qv_ps = ps2.tile([128, 128], F32, tag="qv_ps")
                    nc.tensor.matmul(qv_ps[:], lhsT=attT[:], rhs=wsb["wm"][:], start=True, stop=True)
                    msg = p2s.tile([128, 128], BF16, tag="msg")
                    nc.vector.tensor_tensor(msg[:], x0_res[:, ts(t, 128)], qv_ps[:], op=OP.add)

                    # mlp
                    msgT_ps = ps2.tile([128, 128], BF16, tag="msgT_ps")
                    msgT = p2s.tile([128, 128], BF16, tag="msgT")
                    nc.tensor.transpose(msgT_ps[:], msg[:], ident[:])
                    nc.scalar.copy(msgT[:], msgT_ps[:])
                    m1_ps = ps2.tile([128, 128], F32, tag="m1_ps")
                    nc.tensor.matmul(m1_ps[:], lhsT=msgT[:], rhs=wsb["w1"][:], start=True, stop=True)
                    m1 = p2s.tile([128, 128], BF16, tag="m1")
                    nc.scalar.activation(m1[:], m1_ps[:], AF.Relu)
                    m1T_ps = ps2.tile([128, 128], BF16, tag="m1T_ps")
                    m1T = p2s.tile([128, 128], BF16, tag="m1T")
                    nc.tensor.transpose(m1T_ps[:], m1[:], ident[:])
                    nc.scalar.copy(m1T[:], m1T_ps[:])
                    m2_ps = ps2.tile([128, 128], F32, tag="m2_ps")
                    nc.tensor.matmul(m2_ps[:], lhsT=m1T[:], rhs=wsb["w2"][:], start=True, stop=True)

                    # LN2 + residual
                    m2sb = p2s.tile([128, 128], F32, tag="m2sb")
                    nc.scalar.activation(m2sb[:], m2_ps[:], AF.Copy)
                    ssum = p2s.tile([128, 1], F32, tag="ssum2")
                    mu = p2s.tile([128, 1], F32, tag="mu2")
                    sq = p2s.tile([128, 128], F32, tag="sq2")
                    m2st = p2s.tile([128, 1], F32, tag="m2st")
                    var = p2s.tile([128, 1], F32, tag="var2")
                    std = p2s.tile([128, 1], F32, tag="std2")
                    rstd = p2s.tile([128, 1], F32, tag="rstd2")
                    nc.scalar.activation(sq[:], m2sb[:], AF.Square, accum_out=m2st[:])
                    nc.scalar.activation(sqc2 := p2s.tile([128, 128], BF16, tag="sqc2", name="sqc2"),
                                         m2sb[:], AF.Copy, accum_out=ssum[:])
                    nc.vector.tensor_scalar_mul(mu[:], ssum[:], 1.0 / C)
                    nc.vector.tensor_scalar(
                        var[:], mu[:], scalar1=mu[:], scalar2=None, op0=OP.mult
                    )
                    nc.vector.tensor_scalar(
                        m2s2b := p2s.tile([128, 1], F32, tag="m2s2b", name="m2s2b"),
                        m2st[:], scalar1=1.0 / C, scalar2=EPS, op0=OP.mult, op1=OP.add,
                    )
                    nc.vector.tensor_tensor(var[:], m2s2b, var[:], op=OP.subtract)
                    nc.scalar.activation(std[:], var[:], AF.Sqrt)
                    nc.vector.reciprocal(rstd[:], std[:])
                    zn = p2s.tile([128, 128], F32, tag="zn")
                    outt = p2s.tile([128, 128], F32, tag="outt")
                    nc.vector.tensor_scalar(
                        zn[:], m2sb[:], scalar1=mu[:], scalar2=rstd[:],
                        op0=OP.subtract, op1=OP.mult,
                    )
                    if nontrivial_ln2:
                        nc.vector.tensor_tensor(
                            zn[:], zn[:], g2b2_sb[:, 0:C].to_broadcast([128, C]), op=OP.mult)
                        nc.vector.tensor_tensor(
                            zn[:], zn[:], g2b2_sb[:, C:2 * C].to_broadcast([128, C]), op=OP.add)
                    nc.vector.tensor_tensor(outt[:], zn[:], x0_res[:, ts(t, 128)], op=OP.add)
                    nc.sync.dma_start(out[ts(t, 128), :], outt[:])

    nc.finalize()
    return nc


def prep_core_inputs(x0_img: np.ndarray, query_img: np.ndarray, half: int, w):
    """Host-side prep for one core. w: dict of raw f32 weights g1,b1,g2,b2,Wq..W2."""
    ofs = half * LH
    x0r = np.ascontiguousarray(np.roll(x0_img, -ofs, axis=0))
    lq = query_img[ofs:ofs + LH, :].astype(np.int64)
    lq = (lq - ofs) % L  # remap into rotated coordinates

    gidx32 = np.zeros((128, NT_HALF * NJ), np.int32)
    for t in range(NT_HALF):
        gidx32[:, t * NJ:(t + 1) * NJ] = lq[t * 128:(t + 1) * 128, :]
    bf = ml_dtypes.bfloat16
    g1 = w["g1"]
    m = {
        "x0f": x0r,
        "gidx32": gidx32,
        "ident": np.eye(C, dtype=np.float32).astype(bf),
        "wq": np.ascontiguousarray((0.25 * w["Wq"] * g1[None, :]).T).astype(bf),
        "wk": np.ascontiguousarray((w["Wk"] * g1[None, :]).T).astype(bf),
        "wv": np.ascontiguousarray((w["Wv"] * g1[None, :]).T).astype(bf),
        "wm": np.ascontiguousarray(w["Wm"].T).astype(bf),
        "w1": np.ascontiguousarray(w["W1"].T).astype(bf),
        "w2": np.ascontiguousarray(w["W2"].T).astype(bf),
    }
    b1 = w["b1"]
    nontrivial_ln1 = bool(np.any(b1 != 0.0))
    if nontrivial_ln1:
        m["bqkv"] = np.concatenate(
            [0.25 * (w["Wq"] @ b1), w["Wk"] @ b1, w["Wv"] @ b1]
        ).reshape(1, 3 * C).astype(np.float32)
    nontrivial_ln2 = bool(np.any(w["g2"] != 1.0) or np.any(w["b2"] != 0.0))
    if nontrivial_ln2:
        m["g2b2"] = np.concatenate([w["g2"], w["b2"]]).reshape(1, 2 * C).astype(np.float32)
    return m, nontrivial_ln1, nontrivial_ln2


def kernel(**inputs):
    from concourse.bass_utils import run_bass_kernel_spmd

    x0 = np.asarray(inputs["x0"], np.float32)
    query = np.asarray(inputs["query"])
    w = {k: np.asarray(inputs[k], np.float32)
         for k in ["Wq", "Wk", "Wv", "Wm", "W1", "W2", "g1", "b1", "g2", "b2"]}
    B = x0.shape[0]

    in_maps = []
    nt1 = nt2 = False
    for c in range(8):
        b, half = c // 2, c % 2
        m, nt1, nt2 = prep_core_inputs(x0[b], np.asarray(query[b]), half, w)
        in_maps.append(m)

    nc = build_nc(nt1, nt2)
    res = run_bass_kernel_spmd(nc, in_maps, core_ids=list(range(8)))

    outp = np.empty((B, L, C), np.float32)
    for c in range(8):
        b, half = c // 2, c % 2
        outp[b, half * LH:(half + 1) * LH, :] = res.results[c]["out"]
    return outp

